# revision 1
# baseline (speedup 1.0000x reference)
"""Trainium2 Bass kernel for windowed 3D attention with decomposed rel-pos bias.

Problem: B=4, N=4096 (16^3), C=384, window 8^3=512 tokens, 6 heads x 64 dim.
Sharding: 8 cores, data-parallel over 32 windows (4 per core). Core i takes
batch b=i//2, z-half h=i%2 -> a contiguous [2048, 384] slice of x holding 4
windows (wy, wx in {0,1}).

Device-side per window:
  xT via DMA-transpose; qT/kT produced in a "gapped" 88-row channel layout
  (gaps at rows 0-8/32-40/64-72 hold rel-pos terms / E-indicators so the
  S^T matmul contracts value+bias in ONE pass); S^T = stk^T @ rhsq in PSUM;
  exp on ACT -> P^T bf16; attn@V with a ones-column for the softmax
  denominator; normalize via DMA-broadcast denom + DVE divide; per-head
  projection accumulated in PSUM; bias via rank-1 matmuls.
"""

import os
import numpy as np
import ml_dtypes

BF16 = np.float16

B, N, C = 4, 4096, 384
WS, NH, HD = 8, 6, 64
T = WS * WS * WS  # 512
SCALE = HD ** -0.5
NCORES = 8

# gapped channel layout: 88 contraction rows per head
GAP_SLOTS = [0, 32, 64]                      # x, y, z rel slots (8 rows each)
CHAN_ROWS = list(range(8, 32)) + list(range(40, 64)) + list(range(72, 88))
GROW = 88

_CACHE = {}


def _build_nc():
    import concourse.bass as bass
    import concourse.tile as tile
    import concourse.mybir as mybir
    from contextlib import ExitStack

    f32 = mybir.dt.float32
    bf16 = mybir.dt.float16
    Ident = mybir.ActivationFunctionType.Identity
    Exp = mybir.ActivationFunctionType.Exp
    add = mybir.AluOpType.add
    divide = mybir.AluOpType.divide

    nc = bass.Bass("TRN2")

    x_d = nc.declare_dram_parameter("xt_sh", [4, 128, 3, T], bf16, isOutput=False)
    wqk_d = nc.declare_dram_parameter("wqk_g", [128, 2 * NH * 3 * GROW], bf16, isOutput=False)
    wv_d = nc.declare_dram_parameter("wv", [128, 3 * 390], bf16, isOutput=False)
    bqk_d = nc.declare_dram_parameter("bqk_g", [128, 2 * NH], f32, isOutput=False)

    pw_d = nc.declare_dram_parameter("pw", [64, NH * C], bf16, isOutput=False)
    # misc pack: rtbl [128,192] | eall rows0-24 [*,512] | selg rows0-24 [*,72]
    # | ones row0 [*,128] | vb row0 [*,390] | pb row0 [*,384]
    misc_d = nc.declare_dram_parameter("misc", [128, 192 + 512 + 72 + 128 + 390 + 384], bf16, isOutput=False)
    out_d = nc.declare_dram_parameter("out_w", [4, T, C], f32, isOutput=True)

    with tile.TileContext(nc) as tc, ExitStack() as ctx:
        const = ctx.enter_context(tc.tile_pool(name="const", bufs=1))
        xnat_p = ctx.enter_context(tc.tile_pool(name="xnat", bufs=2))
        xt_p = ctx.enter_context(tc.tile_pool(name="xt", bufs=3))
        vaug_p = ctx.enter_context(tc.tile_pool(name="vaug", bufs=3))
        pp_p = ctx.enter_context(tc.tile_pool(name="pp", bufs=4))
        osb_p = ctx.enter_context(tc.tile_pool(name="osb", bufs=12))
        rd_p = ctx.enter_context(tc.tile_pool(name="rd", bufs=4))
        ost_p = ctx.enter_context(tc.tile_pool(name="ost", bufs=4))
        qkp = ctx.enter_context(tc.tile_pool(name="qkp", bufs=2, space="PSUM"))
        stp = ctx.enter_context(tc.tile_pool(name="stp", bufs=2, space="PSUM"))
        mip = ctx.enter_context(tc.tile_pool(name="mip", bufs=2, space="PSUM"))

        # --- load constants ---
        wqk_sb = const.tile([128, 2 * NH * 3 * GROW], bf16)
        nc.sync.dma_start(out=wqk_sb, in_=wqk_d[:, :])
        wq_sb = wqk_sb[:, 0:NH * 3 * GROW]
        wk_sb = wqk_sb[:, NH * 3 * GROW:]
        wv_sb = const.tile([128, 3 * 390], bf16)
        nc.sync.dma_start(out=wv_sb, in_=wv_d[:, :])
        bqk_sb = const.tile([128, 2 * NH], f32)
        nc.sync.dma_start(out=bqk_sb, in_=bqk_d[:, :])
        bq_sb = bqk_sb[:, 0:NH]
        bk_sb = bqk_sb[:, NH:]
        pw_sb = const.tile([64, NH * C], bf16)
        nc.sync.dma_start(out=pw_sb, in_=pw_d[:, :])
        misc_sb = const.tile([128, 192 + 512 + 72 + 128 + 390 + 384], bf16)
        nc.sync.dma_start(out=misc_sb, in_=misc_d[:, :])
        rtbl_sb = misc_sb[:, 0:192]
        eall_sb = misc_sb[:, 192:704]
        selg_sb = misc_sb[:, 704:776]
        ones1_sb = misc_sb[:, 776:904]
        vb_sb = misc_sb[:, 904:1294]
        pb_sb = misc_sb[:, 1294:1678]

        # persistent per-head tiles: stk (lhsT side: E rows + kT) and rhsq
        # (rhs side: Rel rows + qT)
        stk = [const.tile([128, T], bf16, name=f"stk{h}", tag=f"stk{h}") for h in range(NH)]
        rhsq = [const.tile([128, T], bf16, name=f"rhsq{h}", tag=f"rhsq{h}") for h in range(NH)]

        for w in range(4):
            # --- load pre-transposed x window ---
            xt = xt_p.tile([128, 3, T], bf16)
            for cc in range(3):
                nc.sync.dma_start(out=xt[:, cc, :], in_=x_d[w, :, cc, :])

            # --- v in natural token layout, 65-strided with ones column ---
            va = vaug_p.tile([128, 4, NH, 65], bf16)
            for ct in range(4):
                vp = mip.tile([128, 512], f32, tag="mi", name="vp")
                for cc in range(3):
                    nc.tensor.matmul(
                        vp[:, 0:390],
                        lhsT=xt[:, cc, 128 * ct:128 * ct + 128],
                        rhs=wv_sb[:, 390 * cc:390 * cc + 390],
                        start=(cc == 0), stop=False,
                    )
                nc.tensor.matmul(
                    vp[:, 0:390], lhsT=ones1_sb[0:1, :], rhs=vb_sb[0:1, :],
                    start=False, stop=True,
                )
                nc.vector.tensor_copy(
                    va[:, ct].rearrange("p h x -> p (h x)"), vp[:, 0:390]
                )

            osb_list = []
            for h in range(NH):
                # --- q side: gapped channels + rel terms in the gaps ---
                qp = qkp.tile([128, T], f32, tag="qk", name="qp")
                for cc in range(3):
                    nc.tensor.matmul(
                        qp[0:GROW, :],
                        lhsT=wq_sb[:, (h * 3 + cc) * GROW:(h * 3 + cc + 1) * GROW],
                        rhs=xt[:, cc, :],
                        start=(cc == 0), stop=(cc == 2),
                    )
                # evac 1: biased qT -> rhsq (gap rows become 0)
                nc.vector.tensor_scalar(
                    out=rhsq[h][0:GROW, :], in0=qp[0:GROW, :],
                    scalar1=bq_sb[0:GROW, h:h + 1], scalar2=None, op0=add,
                )
                # rel-pos group matmuls accumulate into the psum gap rows
                rq3 = rhsq[h].rearrange("p (z y x) -> p z y x", z=8, y=8, x=8)
                qp3 = qp.rearrange("p (z y x) -> p z y x", z=8, y=8, x=8)
                for a in range(3):
                    for g in range(8):
                        lhsT = rtbl_sb[0:GROW, (a * 8 + g) * 8:(a * 8 + g + 1) * 8]
                        if a == 0:
                            rhs_ap = rq3[0:GROW, g, :, :]
                            out_ap = qp[0:8, 64 * g:64 * g + 64]
                        elif a == 1:
                            rhs_ap = rq3[0:GROW, :, g, :]
                            out_ap = qp3[32:40, :, g, :]
                        else:
                            rhs_ap = rq3[0:GROW, :, :, g]
                            out_ap = qp3[64:72, :, :, g]
                        # strided psum outs: token-order directly (walrus OK;
                        # CoreSim interp can't model these — validated on hw)
                        nc.tensor.matmul(
                            out_ap, lhsT=lhsT, rhs=rhs_ap,
                            start=(g == 0), stop=(g == 7),
                        )
                # evac 2 (ACT): rewrite chan rows, pick up rel rows
                nc.scalar.activation(
                    out=rhsq[h][0:GROW, :], in_=qp[0:GROW, :],
                    func=Ident, bias=bq_sb[0:GROW, h:h + 1],
                )


                # --- k side: gapped channels + E indicator rows ---
                kp = qkp.tile([128, T], f32, tag="qk", name="kp")
                for cc in range(3):
                    nc.tensor.matmul(
                        kp[0:GROW, :],
                        lhsT=wk_sb[:, (h * 3 + cc) * GROW:(h * 3 + cc + 1) * GROW],
                        rhs=xt[:, cc, :],
                        start=(cc == 0), stop=(cc == 2),
                    )
                    if cc == 0:
                        # E indicator rows into the gap slots (same psum group)
                        nc.tensor.matmul(
                            kp[0:72, :], lhsT=selg_sb[0:24, 0:72],
                            rhs=eall_sb[0:24, :], start=False, stop=False,
                        )
                nc.scalar.activation(
                    out=stk[h][0:GROW, :], in_=kp[0:GROW, :],
                    func=Ident, bias=bk_sb[0:GROW, h:h + 1],
                )

                # --- S^T (+bias) -> exp -> attn@V ---
                ot = mip.tile([128, 512], f32, tag="mi", name="ot")
                for pair in range(2):
                    stt = stp.tile([128, 1024], f32, tag="stt", name="stt")
                    for j in range(2):
                        kc = 2 * pair + j
                        nc.tensor.matmul(
                            stt[:, 512 * j:512 * j + 512],
                            lhsT=stk[h][0:GROW, 128 * kc:128 * kc + 128],
                            rhs=rhsq[h][0:GROW, :],
                            start=True, stop=True,
                        )
                    pp = pp_p.tile([128, 1024], bf16)
                    nc.scalar.activation(out=pp[:, :], in_=stt[:, :], func=Exp)
                    for j in range(2):
                        kc = 2 * pair + j
                        nc.tensor.matmul(
                            ot[0:65, :],
                            lhsT=va[:, kc, h, :],
                            rhs=pp[:, 512 * j:512 * j + 512],
                            start=(kc == 0), stop=(kc == 3),
                        )

                # --- normalize: evacuate OT, recip denom row, matmul-bcast
                # recip into rows 64:128 of the psum bank, multiply ---
                osb_un = osb_p.tile([128, T], bf16, tag="osb_un", name="osb_un")
                nc.vector.tensor_copy(osb_un[0:64, :], ot[0:64, :])
                rdr = rd_p.tile([1, 512], bf16, tag="rdr", name="rdr")
                with nc.allow_low_precision(reason="softmax denom recip in fp16"):
                    nc.vector.reciprocal(rdr[0:1, :], ot[64:65, :])
                nc.tensor.matmul(
                    ot[64:128, :], lhsT=ones1_sb[0:1, 0:64], rhs=rdr[0:1, :],
                    start=True, stop=True,
                )
                osb = osb_p.tile([128, T], bf16)
                nc.vector.tensor_mul(osb[0:64, :], osb_un[0:64, :], ot[64:128, :])
                osb_list.append(osb)

            # --- projection: accumulate heads per q-chunk ---
            for qc in range(4):
                prj = mip.tile([128, 512], f32, tag="mi", name="prj")
                for h in range(NH):
                    nc.tensor.matmul(
                        prj[:, 0:C],
                        lhsT=osb_list[h][0:64, 128 * qc:128 * qc + 128],
                        rhs=pw_sb[0:64, C * h:C * h + C],
                        start=(h == 0), stop=False,
                    )
                nc.tensor.matmul(
                    prj[:, 0:C], lhsT=ones1_sb[0:1, :], rhs=pb_sb[0:1, :],
                    start=False, stop=True,
                )
                ost = ost_p.tile([128, C], f32)
                nc.vector.tensor_copy(ost[:, :], prj[:, 0:C])
                nc.sync.dma_start(
                    out=out_d[w, 128 * qc:128 * qc + 128, :], in_=ost[:, :]
                )

    _fix_multiwait(nc)
    return nc


def _fix_multiwait(nc):
    """Walrus in this container rejects instructions with >1 sync wait.
    Move extra waits onto same-engine NOPs inserted just before."""
    import bass_rust
    import concourse.mybir as mybir

    eng_map = {}
    for eng in (nc.tensor, nc.vector, nc.scalar, nc.gpsimd, nc.sync):
        eng_map[eng.engine] = eng

    f = nc.m.functions[0]
    blocks = list(f.blocks)

    def make_nop(engine_type, wait):
        eng = eng_map[engine_type]
        bi = eng.nop()
        mi = bi.ins
        mi.sync_info = bass_rust.SyncInfo(on_wait=[wait], on_update=[])
        # remove from wherever bass appended it
        for b in blocks:
            bl = b.instructions
            for j in range(len(bl) - 1, -1, -1):
                if bl[j] is mi:
                    del bl[j]
                    return mi
        raise RuntimeError("nop not found after emission")

    for blk in blocks:
        insts = blk.instructions       # live list
        out = []
        changed = False
        for i in insts:
            si = i.sync_info
            if si is not None and len(si.on_wait) > 1:
                waits = list(si.on_wait)
                for w in waits[:-1]:
                    out.append(make_nop(i.engine, w))
                i.sync_info = bass_rust.SyncInfo(
                    on_wait=[waits[-1]], on_update=list(si.on_update)
                )
                changed = True
            out.append(i)
        if changed:
            insts[:] = out


def _host_prep(x, qkv_w, qkv_b, proj_w, proj_b, rel_pos_x, rel_pos_y, rel_pos_z):
    """Build the shared (replicated) device arrays from the raw inputs."""
    qkv_w = np.asarray(qkv_w, np.float32)
    qkv_b = np.asarray(qkv_b, np.float32)
    proj_w = np.asarray(proj_w, np.float32)
    proj_b = np.asarray(proj_b, np.float32)
    rels = [np.asarray(r, np.float32) for r in (rel_pos_x, rel_pos_y, rel_pos_z)]

    cr = np.array(CHAN_ROWS)

    def gapped_w(Wm, scale):
        # Wm [384, 384] -> [128, NH*3*88] lhsT layout
        G = np.zeros((C, NH, GROW), np.float32)
        for h in range(NH):
            G[:, h, cr] = Wm[:, 64 * h:64 * h + 64] * scale
        return np.ascontiguousarray(
            G.reshape(3, 128, NH, GROW).transpose(1, 2, 0, 3).reshape(128, NH * 3 * GROW)
        ).astype(BF16)

    def gapped_b(bm, scale):
        Gb = np.zeros((128, NH), np.float32)
        for h in range(NH):
            Gb[cr, h] = bm[64 * h:64 * h + 64] * scale
        return Gb

    wq_g = gapped_w(qkv_w[:, 0:C], SCALE)
    wk_g = gapped_w(qkv_w[:, C:2 * C], 1.0)
    bq_g = gapped_b(qkv_b[0:C], SCALE)
    bk_g = gapped_b(qkv_b[C:2 * C], 1.0)
    Wv = qkv_w[:, 2 * C:]
    wv_aug = np.zeros((C, 3, NH, 65), np.float32)
    wv_aug[:, :, :, :] = 0.0
    for h in range(NH):
        wv_aug[:, 0, h, 0:64] = 0.0
    Wv3 = Wv.reshape(C, NH, 64)
    wva = np.zeros((C, NH, 65), np.float32)
    wva[:, :, 0:64] = Wv3
    wv = np.ascontiguousarray(
        wva.reshape(3, 128, NH * 65).transpose(1, 0, 2).reshape(128, 3 * 390)
    ).astype(BF16)
    vba = np.zeros((1, NH, 65), np.float32)
    vba[0, :, 0:64] = qkv_b[2 * C:].reshape(NH, 64)
    vba[0, :, 64] = 1.0
    vb = vba.reshape(1, 390).astype(BF16)
    pb = proj_b.reshape(1, C).astype(BF16)
    pw = np.zeros((64, NH * C), np.float32)
    for h in range(NH):
        pw[:, C * h:C * h + C] = proj_w[64 * h:64 * h + 64, :]
    pw = pw.astype(BF16)

    # rel tables: rtbl[chan_row(c), (a*8+g)*8 + dk'] = Ra[g - dk' + 7, c] / SCALE
    # (the rel matmuls consume the already-scaled qT, reference uses unscaled q)
    rtbl = np.zeros((128, 3 * 8 * 8), np.float32)
    for a in range(3):
        Ra = rels[a]  # [15, 64]
        for g in range(8):
            for dk in range(8):
                rtbl[cr, (a * 8 + g) * 8 + dk] = Ra[g - dk + 7, :] / SCALE
    rtbl = rtbl.astype(BF16)

    # E indicators [24, 512]; k = 64*dk + 8*hk + wk
    k_idx = np.arange(T)
    dk, hk, wk = k_idx >> 6, (k_idx >> 3) & 7, k_idx & 7
    eall = np.zeros((24, T), np.float32)
    for cpr in range(8):
        eall[cpr, :] = (dk == cpr)
        eall[8 + cpr, :] = (hk == cpr)
        eall[16 + cpr, :] = (wk == cpr)
    eall = eall.astype(BF16)

    selg = np.zeros((24, 72), np.float32)
    for a in range(3):
        for cpr in range(8):
            selg[8 * a + cpr, 32 * a + cpr] = 1.0
    selg = selg.astype(BF16)

    misc = np.zeros((128, 192 + 512 + 72 + 128 + 390 + 384), BF16)
    misc[:, 0:192] = rtbl
    misc[0:24, 192:704] = eall
    misc[0:24, 704:776] = selg
    misc[0:1, 776:904] = 1.0
    misc[0:1, 904:1294] = vb
    misc[0:1, 1294:1678] = pb
    return dict(
        wqk_g=np.concatenate([wq_g, wk_g], axis=1),
        wv=wv,
        bqk_g=np.concatenate([bq_g, bk_g], axis=1).astype(np.float32),
        pw=pw, misc=misc,
    )


LAST_EXEC_NS = None


def kernel(**inputs) -> np.ndarray:
    global LAST_EXEC_NS
    from concourse.bass_utils import run_bass_kernel_spmd

    if "nc" not in _CACHE:
        _CACHE["nc"] = _build_nc()
    nc = _CACHE["nc"]

    x = np.asarray(inputs["x"], np.float32)
    shared = _host_prep(
        x, inputs["qkv_w"], inputs["qkv_b"], inputs["proj_w"], inputs["proj_b"],
        inputs["rel_pos_x"], inputs["rel_pos_y"], inputs["rel_pos_z"],
    )

    # window gather indices within a [2048, C] shard (4 windows x 512 tokens)
    t = np.arange(T)
    z, yy, xx = t >> 6, (t >> 3) & 7, t & 7
    rows_w = np.stack([
        256 * z + 16 * (8 * (w >> 1) + yy) + (8 * (w & 1) + xx) for w in range(4)
    ])  # [4, 512]

    in_maps = []
    for i in range(NCORES):
        b, half = i // 2, i % 2
        m = dict(shared)
        xs = x[b, half * 2048:(half + 1) * 2048, :]          # [2048, C]
        xw = xs[rows_w, :]                                    # [4, 512, C]
        xt4 = xw.transpose(0, 2, 1).reshape(4, 3, 128, T)
        m["xt_sh"] = np.ascontiguousarray(
            xt4.transpose(0, 2, 1, 3)
        ).astype(BF16)                                        # [4, 128, 3, 512]
        in_maps.append(m)

    trace = bool(os.environ.get("KERNEL_TRACE"))
    try:
        res = run_bass_kernel_spmd(
            nc, in_maps, core_ids=list(range(NCORES)), trace=trace,
        )
    except (ModuleNotFoundError, ImportError):
        # NTFF profile hook unavailable in this container - run untraced
        res = run_bass_kernel_spmd(
            nc, in_maps, core_ids=list(range(NCORES)), trace=False,
        )
    LAST_EXEC_NS = res.exec_time_ns

    out = np.empty((B, N, C), np.float32)
    for i in range(NCORES):
        b, half = i // 2, i % 2
        ow = res.results[i]["out_w"]                          # [4, 512, C]
        sh = np.empty((2048, C), np.float32)
        sh[rows_w.reshape(-1), :] = ow.reshape(4 * T, C)
        out[b, half * 2048:(half + 1) * 2048, :] = sh
    return out.reshape(B, N, C)



# revision 19
# speedup vs baseline: 1.1538x; 1.1538x over previous
"""Trainium2 Bass kernel for windowed 3D attention with decomposed rel-pos bias.

Problem: B=4, N=4096 (16^3), C=384, window 8^3=512 tokens, 6 heads x 64 dim.
Sharding: 8 cores, data-parallel over 32 windows (4 per core). Core i takes
batch b=i//2, z-half h=i%2 -> a contiguous [2048, 384] slice of x holding 4
windows.

Device-side per window (v2, fp8-DoubleRow S^T):
  - q/k projections 2-head-packed: one [128,512] PSUM tile per head-pair per
    contraction chunk (out rows 0:64 = even head, 64:128 = odd head).
  - S^T contraction operands stored fp8e4m3 in DoubleRow layout [64,2,512]:
    ktile0 = 64 channel rows, ktile1 rows 0:24 = rel-pos terms (q side) /
    one-hot E indicators (k side, static), rows 24:64 zero. One DoubleRow
    matmul per (head, k-chunk) computes S^T at 0.5 cycles/col.
  - exp on ACT (f32 PSUM -> bf16 SBUF), attn@V in bf16 with a ones column
    producing the softmax denominator row.
  - normalization pair-packed: recip denom rows, rank-1 broadcast matmuls,
    one tensor-tensor multiply per pair.
  - output projection contracts a full head-pair (128 rows) per matmul;
    v/proj biases folded into PSUM-evac tensor-tensor adds against
    host-broadcast bias tiles (no bias matmuls).
"""

import os
import numpy as np
import ml_dtypes

BF16 = np.float16            # host array dtype for mybir float16 params
FP8 = ml_dtypes.float8_e4m3fn

B, N, C = 4, 4096, 384
WS, NH, HD = 8, 6, 64
T = WS * WS * WS  # 512
SCALE = HD ** -0.5
NCORES = 8
NP = NH // 2  # head pairs

_CACHE = {}


def _build_nc():
    import concourse.bass as bass
    import concourse.tile as tile
    import concourse.mybir as mybir
    from contextlib import ExitStack

    f32 = mybir.dt.float32
    bf16 = mybir.dt.float16
    fp8 = mybir.dt.float8e4
    Exp = mybir.ActivationFunctionType.Exp
    Copy = mybir.ActivationFunctionType.Copy
    Ident = mybir.ActivationFunctionType.Identity
    add = mybir.AluOpType.add
    mult = mybir.AluOpType.mult
    DR = mybir.MatmulPerfMode.DoubleRow

    nc = bass.Bass("TRN2")

    x_d = nc.declare_dram_parameter("xt_sh", [4, 128, 3, T], bf16, isOutput=False)
    wqk_d = nc.declare_dram_parameter("wqk", [128, 18 * 128], bf16, isOutput=False)
    bqk_d = nc.declare_dram_parameter("bqk", [128, 6], f32, isOutput=False)
    wv_d = nc.declare_dram_parameter("wv", [128, 3 * 390], bf16, isOutput=False)
    vb_d = nc.declare_dram_parameter("vb", [1, 390], bf16, isOutput=False)
    pw_d = nc.declare_dram_parameter("pw", [128, 3 * 384], bf16, isOutput=False)
    pbb_d = nc.declare_dram_parameter("pbb", [128, 384], bf16, isOutput=False)
    rtbl_d = nc.declare_dram_parameter("rtbl8", [64, 768], fp8, isOutput=False)
    stki_d = nc.declare_dram_parameter("stk8i", [72, NH * 2 * T], fp8, isOutput=False)
    rhqi_d = nc.declare_dram_parameter("rhq8i", [72, NH * 2 * T], fp8, isOutput=False)
    ones_d = nc.declare_dram_parameter("ones1", [1, 128], bf16, isOutput=False)
    out_d = nc.declare_dram_parameter("out_w", [4, T, C], f32, isOutput=True)

    with tile.TileContext(nc) as tc, ExitStack() as ctx:
        const = ctx.enter_context(tc.tile_pool(name="const", bufs=1))
        xt_p = ctx.enter_context(tc.tile_pool(name="xt", bufs=3))
        vaug_p = ctx.enter_context(tc.tile_pool(name="vaug", bufs=3))
        pp_p = ctx.enter_context(tc.tile_pool(name="pp", bufs=4))
        pf_p = ctx.enter_context(tc.tile_pool(name="pf", bufs=6))
        rd_p = ctx.enter_context(tc.tile_pool(name="rd", bufs=4))
        ost_p = ctx.enter_context(tc.tile_pool(name="ost", bufs=6))
        qkp = ctx.enter_context(tc.tile_pool(name="qkp", bufs=2, space="PSUM"))
        stp = ctx.enter_context(tc.tile_pool(name="stp", bufs=2, space="PSUM"))
        mip = ctx.enter_context(tc.tile_pool(name="mip", bufs=2, space="PSUM"))

        # --- first window's x plus constants, ordered by first use; the big
        # weight / init tensors load on the ACT queue in parallel with SP ---
        xt0 = xt_p.tile([128, 3, T], bf16)
        for cc in range(3):
            nc.sync.dma_start(out=xt0[:, cc, :], in_=x_d[0, :, cc, :])
        wv_sb = const.tile([128, 3 * 390], bf16)
        nc.sync.dma_start(out=wv_sb, in_=wv_d[:, :])
        vb_sb = const.tile([1, 390], bf16)
        nc.sync.dma_start(out=vb_sb, in_=vb_d[:, :])
        bqk_sb = const.tile([128, 6], f32)
        nc.sync.dma_start(out=bqk_sb, in_=bqk_d[:, :])
        rtbl_sb = const.tile([64, 768], fp8)
        nc.sync.dma_start(out=rtbl_sb, in_=rtbl_d[:, :])
        ones_sb = const.tile([1, 128], bf16)
        nc.sync.dma_start(out=ones_sb, in_=ones_d[:, :])

        wqk_sb = const.tile([128, 18 * 128], bf16)
        nc.scalar.dma_start(out=wqk_sb, in_=wqk_d[:, :])
        pw_sb = const.tile([128, 3 * 384], bf16)
        nc.scalar.dma_start(out=pw_sb, in_=pw_d[:, :])
        pbb_sb = const.tile([128, 384], bf16)
        nc.scalar.dma_start(out=pbb_sb, in_=pbb_d[:, :])

        # persistent DoubleRow operand tiles [72, 2, 512] fp8 per head:
        # ktile0 rows 0:64 = channels (rows 64:72 zero-pad), ktile1 = rel/E at
        # slots 0:8 / 32:40 / 64:72 with zeros between (legal PSUM out bases)
        stk8bs, rhq8bs = [], []
        for bi in range(2):
            sb = const.tile([72, NH, 2, T], fp8, name=f"stk8b{bi}", tag=f"stk8b{bi}")
            nc.sync.dma_start(
                out=sb.rearrange("p h t x -> p (h t x)"), in_=stki_d[:, :])
            rb = const.tile([72, NH, 2, T], fp8, name=f"rhq8b{bi}", tag=f"rhq8b{bi}")
            nc.sync.dma_start(
                out=rb.rearrange("p h t x -> p (h t x)"), in_=rhqi_d[:, :])
            stk8bs.append(sb)
            rhq8bs.append(rb)

        def wq_ap(side, pair, cc):
            i = (side * 9 + pair * 3 + cc) * 128
            return wqk_sb[:, i:i + 128]

        for w in range(4):
            stk8b = stk8bs[w % 2]
            rhq8b = rhq8bs[w % 2]
            if w == 0:
                xt = xt0
            else:
                xt = xt_p.tile([128, 3, T], bf16)
                for cc in range(3):
                    nc.sync.dma_start(out=xt[:, cc, :], in_=x_d[w, :, cc, :])

            # --- v in token layout, 65-strided, bias+ones via Pool TT-add ---
            va = vaug_p.tile([128, 4, 390], bf16)
            for ct in range(4):
                vp = mip.tile([128, 512], f32, tag="mi", name="vp")
                for cc in range(3):
                    nc.tensor.matmul(
                        vp[:, 0:390],
                        lhsT=xt[:, cc, 128 * ct:128 * ct + 128],
                        rhs=wv_sb[:, 390 * cc:390 * cc + 390],
                        start=(cc == 0), stop=False,
                    )
                nc.tensor.matmul(
                    vp[:, 0:390], lhsT=ones_sb[0:1, :], rhs=vb_sb[0:1, :],
                    start=False, stop=True,
                )
                nc.scalar.activation(
                    out=va[:, ct, :], in_=vp[:, 0:390], func=Copy)

            # --- q/k projections, 2-head-packed; fp8 DoubleRow operands ---
            for p in range(NP):
                h0, h1 = 2 * p, 2 * p + 1
                qp = qkp.tile([128, 512], f32, tag="qk", name="qp")
                for cc in range(3):
                    nc.tensor.matmul(
                        qp[:, :], lhsT=wq_ap(0, p, cc), rhs=xt[:, cc, :],
                        start=(cc == 0), stop=(cc == 2),
                    )
                nc.vector.tensor_scalar(
                    out=rhq8b[0:64, h0, 0, :], in0=qp[0:64, :],
                    scalar1=bqk_sb[0:64, p:p + 1], scalar2=None, op0=add,
                )
                nc.vector.tensor_scalar(
                    out=rhq8b[0:64, h1, 0, :], in0=qp[64:128, :],
                    scalar1=bqk_sb[64:128, p:p + 1], scalar2=None, op0=add,
                )
                # rel-pos terms for each head of the pair; each (a, g) matmul
                # writes a 32-row block (8 rel rows + 24 table-zero rows) so
                # psum rows 0:96 are all written and one evac covers 0:72
                for h in (h0, h1):
                    rp = qkp.tile([128, 512], f32, tag="qk", name="rp")
                    rq3 = rhq8b.rearrange(
                        "p h t (z y x) -> p h t z y x", z=8, y=8, x=8)
                    rp3 = rp.rearrange("p (z y x) -> p z y x", z=8, y=8, x=8)
                    for a in range(3):
                        for g in range(8):
                            lhsT = rtbl_sb[:, (a * 8 + g) * 32:(a * 8 + g + 1) * 32]
                            if a == 0:
                                rhs_ap = rq3[0:64, h, 0, g, :, :]
                                out_ap = rp[0:32, 64 * g:64 * g + 64]
                            elif a == 1:
                                rhs_ap = rq3[0:64, h, 0, :, g, :]
                                out_ap = rp3[32:64, :, g, :]
                            else:
                                rhs_ap = rq3[0:64, h, 0, :, :, g]
                                out_ap = rp3[64:96, :, :, g]
                            nc.tensor.matmul(
                                out_ap, lhsT=lhsT, rhs=rhs_ap,
                                start=(g == 0), stop=(g == 7),
                            )
                    nc.vector.tensor_copy(rhq8b[0:72, h, 1, :], rp[0:72, :])

                kp = qkp.tile([128, 512], f32, tag="qk", name="kp")
                for cc in range(3):
                    nc.tensor.matmul(
                        kp[:, :], lhsT=wq_ap(1, p, cc), rhs=xt[:, cc, :],
                        start=(cc == 0), stop=(cc == 2),
                    )
                nc.scalar.activation(
                    out=stk8b[0:64, h0, 0, :], in_=kp[0:64, :],
                    func=Ident, bias=bqk_sb[0:64, 3 + p:4 + p],
                )
                nc.scalar.activation(
                    out=stk8b[0:64, h1, 0, :], in_=kp[64:128, :],
                    func=Ident, bias=bqk_sb[64:128, 3 + p:4 + p],
                )

            # --- attention per pair ---
            pf_list = []
            for p in range(NP):
                rdr = [rd_p.tile([1, T], bf16, tag="rdr", name="rdr")
                       for _ in range(2)]
                ots = []
                for j, h in enumerate((2 * p, 2 * p + 1)):
                    ot = mip.tile([128, 512], f32, tag="mi", name="ot")
                    for half in range(2):
                        stt = stp.tile([128, 1024], f32, tag="stt", name="stt")
                        for jj in range(2):
                            kc = 2 * half + jj
                            nc.tensor.matmul(
                                stt[:, 512 * jj:512 * jj + 512],
                                lhsT=stk8b[:, h, :, 128 * kc:128 * kc + 128],
                                rhs=rhq8b[:, h, :, :],
                                start=True, stop=True, perf_mode=DR,
                            )
                        pp = pp_p.tile([128, 1024], bf16)
                        nc.scalar.activation(out=pp[:, :], in_=stt[:, :], func=Exp)
                        for jj in range(2):
                            kc = 2 * half + jj
                            nc.tensor.matmul(
                                ot[0:65, :],
                                lhsT=va[:, kc, 65 * h:65 * h + 65],
                                rhs=pp[:, 512 * jj:512 * jj + 512],
                                start=(kc == 0), stop=(kc == 3),
                            )
                    with nc.allow_low_precision(reason="softmax denom recip"):
                        nc.vector.reciprocal(rdr[j][0:1, :], ot[64:65, :])
                    ots.append(ot)
                # pair-packed normalization
                bc = qkp.tile([128, 512], f32, tag="qk", name="bc")
                nc.tensor.matmul(
                    bc[0:64, :], lhsT=ones_sb[0:1, 0:64], rhs=rdr[0][0:1, :],
                    start=True, stop=True,
                )
                nc.tensor.matmul(
                    bc[64:128, :], lhsT=ones_sb[0:1, 0:64], rhs=rdr[1][0:1, :],
                    start=True, stop=True,
                )
                bcs = pf_p.tile([128, T], bf16, tag="bcs", name="bcs")
                nc.scalar.activation(out=bcs[:, :], in_=bc[:, :], func=Copy)
                pf = pf_p.tile([128, T], bf16, tag="pf", name="pf")
                nc.vector.tensor_tensor(
                    out=pf[0:64, :], in0=ots[0][0:64, :], in1=bcs[0:64, :],
                    op=mult)
                nc.vector.tensor_tensor(
                    out=pf[64:128, :], in0=ots[1][0:64, :], in1=bcs[64:128, :],
                    op=mult)
                pf_list.append(pf)

            # --- output projection: full head-pair contract per matmul ---
            for qc in range(4):
                prj = mip.tile([128, 512], f32, tag="mi", name="prj")
                for p in range(NP):
                    nc.tensor.matmul(
                        prj[:, 0:C],
                        lhsT=pf_list[p][:, 128 * qc:128 * qc + 128],
                        rhs=pw_sb[:, C * p:C * p + C],
                        start=(p == 0), stop=(p == 2),
                    )
                ost = ost_p.tile([128, C], f32)
                nc.vector.tensor_tensor(
                    out=ost[:, :], in0=prj[:, 0:C], in1=pbb_sb[:, :], op=add)
                nc.sync.dma_start(
                    out=out_d[w, 128 * qc:128 * qc + 128, :], in_=ost[:, :])

    _fix_multiwait(nc)
    return nc


def _fix_multiwait(nc):
    """Walrus in this container rejects instructions with >1 sync wait.
    Move extra waits onto same-engine NOPs inserted just before."""
    import bass_rust

    eng_map = {}
    for eng in (nc.tensor, nc.vector, nc.scalar, nc.gpsimd, nc.sync):
        eng_map[eng.engine] = eng

    f = nc.m.functions[0]
    blocks = list(f.blocks)

    def make_nop(engine_type, wait):
        eng = eng_map[engine_type]
        bi = eng.nop()
        mi = bi.ins
        mi.sync_info = bass_rust.SyncInfo(on_wait=[wait], on_update=[])
        for b in blocks:
            bl = b.instructions
            for j in range(len(bl) - 1, -1, -1):
                if bl[j] is mi:
                    del bl[j]
                    return mi
        raise RuntimeError("nop not found after emission")

    for blk in blocks:
        insts = blk.instructions
        out = []
        changed = False
        for i in insts:
            si = i.sync_info
            if si is not None and len(si.on_wait) > 1:
                waits = list(si.on_wait)
                for w in waits[:-1]:
                    out.append(make_nop(i.engine, w))
                i.sync_info = bass_rust.SyncInfo(
                    on_wait=[waits[-1]], on_update=list(si.on_update)
                )
                changed = True
            out.append(i)
        if changed:
            insts[:] = out


def _host_prep(x, qkv_w, qkv_b, proj_w, proj_b, rel_pos_x, rel_pos_y, rel_pos_z):
    """Build the shared (replicated) device arrays from the raw inputs."""
    qkv_w = np.asarray(qkv_w, np.float32)
    qkv_b = np.asarray(qkv_b, np.float32)
    proj_w = np.asarray(proj_w, np.float32)
    proj_b = np.asarray(proj_b, np.float32)
    rels = [np.asarray(r, np.float32) for r in (rel_pos_x, rel_pos_y, rel_pos_z)]

    # wqk [128, (side, pair, cc) x 128]: lhsT chunks, out cols = h0|h1
    wqk = np.zeros((128, 18, 128), np.float32)
    bqk = np.zeros((128, 6), np.float32)
    for side in range(2):
        scale = SCALE if side == 0 else 1.0
        Wm = qkv_w[:, side * C:(side + 1) * C] * scale
        bm = qkv_b[side * C:(side + 1) * C] * scale
        for p in range(NP):
            h0, h1 = 2 * p, 2 * p + 1
            blk = np.concatenate(
                [Wm[:, 64 * h0:64 * h0 + 64], Wm[:, 64 * h1:64 * h1 + 64]],
                axis=1)  # [384, 128]
            for cc in range(3):
                wqk[:, side * 9 + p * 3 + cc, :] = blk[128 * cc:128 * cc + 128, :]
            bqk[0:64, side * 3 + p] = bm[64 * h0:64 * h0 + 64]
            bqk[64:128, side * 3 + p] = bm[64 * h1:64 * h1 + 64]
    wqk = wqk.reshape(128, 18 * 128).astype(BF16)

    # v weights 65-strided with zero ones-column
    Wv3 = qkv_w[:, 2 * C:].reshape(C, NH, 64)
    wva = np.zeros((C, NH, 65), np.float32)
    wva[:, :, 0:64] = Wv3
    wv = np.ascontiguousarray(
        wva.reshape(3, 128, NH * 65).transpose(1, 0, 2).reshape(128, 3 * 390)
    ).astype(BF16)
    vb = np.zeros((NH, 65), np.float32)
    vb[:, 0:64] = qkv_b[2 * C:].reshape(NH, 64)
    vb[:, 64] = 1.0
    vb = vb.reshape(1, 390).astype(BF16)

    # proj weights pair-packed, bias broadcast
    pw = np.zeros((128, 3, C), np.float32)
    for p in range(NP):
        pw[0:64, p, :] = proj_w[64 * (2 * p):64 * (2 * p) + 64, :]
        pw[64:128, p, :] = proj_w[64 * (2 * p + 1):64 * (2 * p + 1) + 64, :]
    pw = pw.reshape(128, 3 * C).astype(BF16)
    pbb = np.broadcast_to(proj_b.reshape(1, C), (128, C)).astype(BF16)

    # rel tables, 32-wide blocks (cols 8:32 zero so the matmul fills the
    # psum rows between rel slots): rtbl8[c, (a*8+g)*32 + j] = Ra[g-j+7, c]/SCALE
    rtbl = np.zeros((64, 768), np.float32)
    for a in range(3):
        Ra = rels[a]
        for g in range(8):
            for j in range(8):
                rtbl[:, (a * 8 + g) * 32 + j] = Ra[g - j + 7, :] / SCALE
    rtbl8 = rtbl.astype(FP8)

    # static DoubleRow init tiles: stk8i has E indicators in ktile1 at
    # slots 0:8 (z-coord), 32:40 (y), 64:72 (x); everything else zero
    k_idx = np.arange(T)
    dk, hk, wk = k_idx >> 6, (k_idx >> 3) & 7, k_idx & 7
    stk8i = np.zeros((72, 2, T), np.float32)
    for cpr in range(8):
        stk8i[cpr, 1, :] = (dk == cpr)
        stk8i[32 + cpr, 1, :] = (hk == cpr)
        stk8i[64 + cpr, 1, :] = (wk == cpr)
    stk8i = np.ascontiguousarray(
        np.broadcast_to(stk8i.reshape(72, 1, 2 * T), (72, NH, 2 * T))
    ).reshape(72, NH * 2 * T).astype(FP8)
    rhq8i = np.zeros((72, NH * 2 * T), FP8)

    ones1 = np.ones((1, 128), BF16)

    return dict(
        wqk=wqk, bqk=bqk, wv=wv, vb=vb, pw=pw, pbb=pbb,
        rtbl8=rtbl8, stk8i=stk8i, rhq8i=rhq8i, ones1=ones1,
    )


LAST_EXEC_NS = None


def kernel(**inputs) -> np.ndarray:
    global LAST_EXEC_NS
    from concourse.bass_utils import run_bass_kernel_spmd

    if "nc" not in _CACHE:
        _CACHE["nc"] = _build_nc()
    nc = _CACHE["nc"]

    x = np.asarray(inputs["x"], np.float32)
    shared = _host_prep(
        x, inputs["qkv_w"], inputs["qkv_b"], inputs["proj_w"], inputs["proj_b"],
        inputs["rel_pos_x"], inputs["rel_pos_y"], inputs["rel_pos_z"],
    )

    # window gather indices within a [2048, C] shard (4 windows x 512 tokens)
    t = np.arange(T)
    z, yy, xx = t >> 6, (t >> 3) & 7, t & 7
    rows_w = np.stack([
        256 * z + 16 * (8 * (w >> 1) + yy) + (8 * (w & 1) + xx) for w in range(4)
    ])  # [4, 512]

    in_maps = []
    for i in range(NCORES):
        b, half = i // 2, i % 2
        m = dict(shared)
        xs = x[b, half * 2048:(half + 1) * 2048, :]          # [2048, C]
        xw = xs[rows_w, :]                                    # [4, 512, C]
        xt4 = xw.transpose(0, 2, 1).reshape(4, 3, 128, T)
        m["xt_sh"] = np.ascontiguousarray(
            xt4.transpose(0, 2, 1, 3)
        ).astype(BF16)                                        # [4, 128, 3, 512]
        in_maps.append(m)

    trace = bool(os.environ.get("KERNEL_TRACE"))
    res = run_bass_kernel_spmd(
        nc, in_maps, core_ids=list(range(NCORES)), trace=trace,
    )
    LAST_EXEC_NS = res.exec_time_ns

    out = np.empty((B, N, C), np.float32)
    for i in range(NCORES):
        b, half = i // 2, i % 2
        ow = res.results[i]["out_w"]                          # [4, 512, C]
        sh = np.empty((2048, C), np.float32)
        sh[rows_w.reshape(-1), :] = ow.reshape(4 * T, C)
        out[b, half * 2048:(half + 1) * 2048, :] = sh
    return out.reshape(B, N, C)


# revision 29
# speedup vs baseline: 1.2924x; 1.1201x over previous
"""Trainium2 Bass kernel for windowed 3D attention with decomposed rel-pos bias.

Problem: B=4, N=4096 (16^3), C=384, window 8^3=512 tokens, 6 heads x 64 dim.
Sharding: 8 cores, data-parallel over 32 windows (4 per core). Core i takes
batch b=i//2, z-half h=i%2 -> a contiguous [2048, 384] slice of x holding 4
windows.

Device-side per window (v2, fp8-DoubleRow S^T):
  - q/k projections 2-head-packed: one [128,512] PSUM tile per head-pair per
    contraction chunk (out rows 0:64 = even head, 64:128 = odd head).
  - S^T contraction operands stored fp8e4m3 in DoubleRow layout [64,2,512]:
    ktile0 = 64 channel rows, ktile1 rows 0:24 = rel-pos terms (q side) /
    one-hot E indicators (k side, static), rows 24:64 zero. One DoubleRow
    matmul per (head, k-chunk) computes S^T at 0.5 cycles/col.
  - exp on ACT (f32 PSUM -> bf16 SBUF), attn@V in bf16 with a ones column
    producing the softmax denominator row.
  - normalization pair-packed: recip denom rows, rank-1 broadcast matmuls,
    one tensor-tensor multiply per pair.
  - output projection contracts a full head-pair (128 rows) per matmul;
    v/proj biases folded into PSUM-evac tensor-tensor adds against
    host-broadcast bias tiles (no bias matmuls).
"""

import os
import numpy as np
import ml_dtypes

BF16 = np.float16            # host array dtype for mybir float16 params
FP8 = ml_dtypes.float8_e4m3fn

B, N, C = 4, 4096, 384
WS, NH, HD = 8, 6, 64
T = WS * WS * WS  # 512
SCALE = HD ** -0.5
NCORES = 8
NP = NH // 2  # head pairs

_CACHE = {}


def _build_nc():
    import concourse.bass as bass
    import concourse.tile as tile
    import concourse.mybir as mybir
    from contextlib import ExitStack

    f32 = mybir.dt.float32
    bf16 = mybir.dt.float16
    fp8 = mybir.dt.float8e4
    Exp = mybir.ActivationFunctionType.Exp
    Copy = mybir.ActivationFunctionType.Copy
    Ident = mybir.ActivationFunctionType.Identity
    add = mybir.AluOpType.add
    mult = mybir.AluOpType.mult
    DR = mybir.MatmulPerfMode.DoubleRow

    nc = bass.Bass("TRN2")

    x_d = nc.declare_dram_parameter("xt_sh", [4, 128, 3, T], bf16, isOutput=False)
    wqk_d = nc.declare_dram_parameter("wqk", [128, 18 * 128], bf16, isOutput=False)
    bqk_d = nc.declare_dram_parameter("bqk", [128, 6], f32, isOutput=False)
    wv_d = nc.declare_dram_parameter("wv", [128, 3 * 390], bf16, isOutput=False)
    vb_d = nc.declare_dram_parameter("vb", [1, 390], bf16, isOutput=False)
    pw_d = nc.declare_dram_parameter("pw", [128, 3 * 384], bf16, isOutput=False)
    pbb_d = nc.declare_dram_parameter("pbb", [128, 384], bf16, isOutput=False)
    rtbl_d = nc.declare_dram_parameter("rtbl8", [64, 768], fp8, isOutput=False)
    stki_d = nc.declare_dram_parameter("stk8i", [72, NH * 2 * T], fp8, isOutput=False)
    rhqi_d = nc.declare_dram_parameter("rhq8i", [72, NH * 2 * T], fp8, isOutput=False)
    ones_d = nc.declare_dram_parameter("ones1", [1, 128], bf16, isOutput=False)
    out_d = nc.declare_dram_parameter("out_w", [4, T, C], f32, isOutput=True)

    with tile.TileContext(nc) as tc, ExitStack() as ctx:
        const = ctx.enter_context(tc.tile_pool(name="const", bufs=1))
        xt_p = ctx.enter_context(tc.tile_pool(name="xt", bufs=3))
        vaug_p = ctx.enter_context(tc.tile_pool(name="vaug", bufs=3))
        pp_p = ctx.enter_context(tc.tile_pool(name="pp", bufs=4))
        pf_p = ctx.enter_context(tc.tile_pool(name="pf", bufs=6))
        rd_p = ctx.enter_context(tc.tile_pool(name="rd", bufs=4))
        ost_p = ctx.enter_context(tc.tile_pool(name="ost", bufs=6))
        qkp = ctx.enter_context(tc.tile_pool(name="qkp", bufs=2, space="PSUM"))
        stp = ctx.enter_context(tc.tile_pool(name="stp", bufs=2, space="PSUM"))
        mip = ctx.enter_context(tc.tile_pool(name="mip", bufs=2, space="PSUM"))

        # --- first window's x plus constants, ordered by first use; the big
        # weight / init tensors load on the ACT queue in parallel with SP ---
        xt0 = xt_p.tile([128, 3, T], bf16)
        for cc in range(3):
            nc.sync.dma_start(out=xt0[:, cc, :], in_=x_d[0, :, cc, :])
        wv_sb = const.tile([128, 3 * 390], bf16)
        nc.sync.dma_start(out=wv_sb, in_=wv_d[:, :])
        vb_sb = const.tile([1, 390], bf16)
        nc.sync.dma_start(out=vb_sb, in_=vb_d[:, :])
        bqk_sb = const.tile([128, 6], f32)
        nc.sync.dma_start(out=bqk_sb, in_=bqk_d[:, :])
        rtbl_sb = const.tile([64, 768], fp8)
        nc.sync.dma_start(out=rtbl_sb, in_=rtbl_d[:, :])
        ones_sb = const.tile([1, 128], bf16)
        nc.sync.dma_start(out=ones_sb, in_=ones_d[:, :])

        wqk_sb = const.tile([128, 18 * 128], bf16)
        nc.scalar.dma_start(out=wqk_sb, in_=wqk_d[:, :])
        pw_sb = const.tile([128, 3 * 384], bf16)
        nc.scalar.dma_start(out=pw_sb, in_=pw_d[:, :])
        pbb_sb = const.tile([128, 384], bf16)
        nc.scalar.dma_start(out=pbb_sb, in_=pbb_d[:, :])

        # persistent DoubleRow operand tiles [72, 2, 512] fp8 per head:
        # ktile0 rows 0:64 = channels (rows 64:72 zero-pad), ktile1 = rel/E at
        # slots 0:8 / 32:40 / 64:72 with zeros between (legal PSUM out bases)
        stk8bs, rhq8bs = [], []
        for bi in range(2):
            sb = const.tile([72, NH, 2, T], fp8, name=f"stk8b{bi}", tag=f"stk8b{bi}")
            nc.sync.dma_start(
                out=sb.rearrange("p h t x -> p (h t x)"), in_=stki_d[:, :])
            rb = const.tile([72, NH, 2, T], fp8, name=f"rhq8b{bi}", tag=f"rhq8b{bi}")
            nc.sync.dma_start(
                out=rb.rearrange("p h t x -> p (h t x)"), in_=rhqi_d[:, :])
            stk8bs.append(sb)
            rhq8bs.append(rb)

        def wq_ap(side, pair, cc):
            i = (side * 9 + pair * 3 + cc) * 128
            return wqk_sb[:, i:i + 128]

        for w in range(4):
            stk8b = stk8bs[w % 2]
            rhq8b = rhq8bs[w % 2]
            if w == 0:
                xt = xt0
            else:
                xt = xt_p.tile([128, 3, T], bf16)
                for cc in range(3):
                    nc.sync.dma_start(out=xt[:, cc, :], in_=x_d[w, :, cc, :])

            # --- v in token layout, 65-strided, bias+ones via Pool TT-add ---
            va = vaug_p.tile([128, 4, 390], bf16)
            for ct in range(4):
                vp = mip.tile([128, 512], f32, tag="mi", name="vp")
                for cc in range(3):
                    nc.tensor.matmul(
                        vp[:, 0:390],
                        lhsT=xt[:, cc, 128 * ct:128 * ct + 128],
                        rhs=wv_sb[:, 390 * cc:390 * cc + 390],
                        start=(cc == 0), stop=False,
                    )
                nc.tensor.matmul(
                    vp[:, 0:390], lhsT=ones_sb[0:1, :], rhs=vb_sb[0:1, :],
                    start=False, stop=True,
                )
                nc.scalar.activation(
                    out=va[:, ct, :], in_=vp[:, 0:390], func=Copy)

            # --- q/k projections, 2-head-packed; fp8 DoubleRow operands ---
            for p in range(NP):
                h0, h1 = 2 * p, 2 * p + 1
                qp = qkp.tile([128, 512], f32, tag="qk", name="qp")
                for cc in range(3):
                    nc.tensor.matmul(
                        qp[:, :], lhsT=wq_ap(0, p, cc), rhs=xt[:, cc, :],
                        start=(cc == 0), stop=(cc == 2),
                    )
                nc.vector.tensor_scalar(
                    out=rhq8b[0:64, h0, 0, :], in0=qp[0:64, :],
                    scalar1=bqk_sb[0:64, p:p + 1], scalar2=None, op0=add,
                )
                nc.vector.tensor_scalar(
                    out=rhq8b[0:64, h1, 0, :], in0=qp[64:128, :],
                    scalar1=bqk_sb[64:128, p:p + 1], scalar2=None, op0=add,
                )
                # rel-pos terms for each head of the pair; each (a, g) matmul
                # writes a 32-row block (8 rel rows + 24 table-zero rows) so
                # psum rows 0:96 are all written and one evac covers 0:72
                for h in (h0, h1):
                    rp = qkp.tile([128, 512], f32, tag="qk", name="rp")
                    rq3 = rhq8b.rearrange(
                        "p h t (z y x) -> p h t z y x", z=8, y=8, x=8)
                    rp3 = rp.rearrange("p (z y x) -> p z y x", z=8, y=8, x=8)
                    for a in range(3):
                        for g in range(8):
                            lhsT = rtbl_sb[:, (a * 8 + g) * 32:(a * 8 + g + 1) * 32]
                            if a == 0:
                                rhs_ap = rq3[0:64, h, 0, g, :, :]
                                out_ap = rp[0:32, 64 * g:64 * g + 64]
                            elif a == 1:
                                rhs_ap = rq3[0:64, h, 0, :, g, :]
                                out_ap = rp3[32:64, :, g, :]
                            else:
                                rhs_ap = rq3[0:64, h, 0, :, :, g]
                                out_ap = rp3[64:96, :, :, g]
                            nc.tensor.matmul(
                                out_ap, lhsT=lhsT, rhs=rhs_ap,
                                start=(g == 0), stop=(g == 7),
                            )
                    nc.vector.tensor_copy(rhq8b[0:72, h, 1, :], rp[0:72, :])

                kp = qkp.tile([128, 512], f32, tag="qk", name="kp")
                for cc in range(3):
                    nc.tensor.matmul(
                        kp[:, :], lhsT=wq_ap(1, p, cc), rhs=xt[:, cc, :],
                        start=(cc == 0), stop=(cc == 2),
                    )
                nc.scalar.activation(
                    out=stk8b[0:64, h0, 0, :], in_=kp[0:64, :],
                    func=Ident, bias=bqk_sb[0:64, 3 + p:4 + p],
                )
                nc.scalar.activation(
                    out=stk8b[0:64, h1, 0, :], in_=kp[64:128, :],
                    func=Ident, bias=bqk_sb[64:128, 3 + p:4 + p],
                )

            # --- attention per pair ---
            pf_list = []
            for p in range(NP):
                rdr = [rd_p.tile([1, T], bf16, tag="rdr", name="rdr")
                       for _ in range(2)]
                ots = []
                for j, h in enumerate((2 * p, 2 * p + 1)):
                    ot = mip.tile([128, 512], f32, tag="mi", name="ot")
                    for half in range(2):
                        stt = stp.tile([128, 1024], f32, tag="stt", name="stt")
                        for jj in range(2):
                            kc = 2 * half + jj
                            nc.tensor.matmul(
                                stt[:, 512 * jj:512 * jj + 512],
                                lhsT=stk8b[:, h, :, 128 * kc:128 * kc + 128],
                                rhs=rhq8b[:, h, :, :],
                                start=True, stop=True, perf_mode=DR,
                            )
                        pp = pp_p.tile([128, 1024], bf16)
                        nc.scalar.activation(out=pp[:, :], in_=stt[:, :], func=Exp)
                        for jj in range(2):
                            kc = 2 * half + jj
                            nc.tensor.matmul(
                                ot[0:65, :],
                                lhsT=va[:, kc, 65 * h:65 * h + 65],
                                rhs=pp[:, 512 * jj:512 * jj + 512],
                                start=(kc == 0), stop=(kc == 3),
                            )
                    with nc.allow_low_precision(reason="softmax denom recip"):
                        nc.vector.reciprocal(rdr[j][0:1, :], ot[64:65, :])
                    ots.append(ot)
                # pair-packed normalization
                bc = stp.tile([128, 512], f32, tag="stt", name="bc")
                nc.tensor.matmul(
                    bc[0:64, :], lhsT=ones_sb[0:1, 0:64], rhs=rdr[0][0:1, :],
                    start=True, stop=True,
                )
                nc.tensor.matmul(
                    bc[64:128, :], lhsT=ones_sb[0:1, 0:64], rhs=rdr[1][0:1, :],
                    start=True, stop=True,
                )
                rdrb = pf_p.tile([128, T], bf16, tag="bcs", name="rdrb")
                nc.vector.tensor_copy(rdrb[:, :], bc[:, :])
                pf = pf_p.tile([128, T], bf16, tag="pf", name="pf")
                nc.vector.tensor_tensor(
                    out=pf[0:64, :], in0=ots[0][0:64, :], in1=rdrb[0:64, :],
                    op=mult)
                nc.vector.tensor_tensor(
                    out=pf[64:128, :], in0=ots[1][0:64, :], in1=rdrb[64:128, :],
                    op=mult)
                pf_list.append(pf)

            # --- output projection: full head-pair contract per matmul ---
            for qc in range(4):
                prj = mip.tile([128, 512], f32, tag="mi", name="prj")
                for p in range(NP):
                    nc.tensor.matmul(
                        prj[:, 0:C],
                        lhsT=pf_list[p][:, 128 * qc:128 * qc + 128],
                        rhs=pw_sb[:, C * p:C * p + C],
                        start=(p == 0), stop=(p == 2),
                    )
                ost = ost_p.tile([128, C], f32)
                nc.vector.tensor_tensor(
                    out=ost[:, :], in0=prj[:, 0:C], in1=pbb_sb[:, :], op=add)
                nc.sync.dma_start(
                    out=out_d[w, 128 * qc:128 * qc + 128, :], in_=ost[:, :])

    _fix_multiwait(nc)
    return nc


def _fix_multiwait(nc):
    """Walrus in this container rejects instructions with >1 sync wait.
    Move extra waits onto same-engine NOPs inserted just before."""
    import bass_rust

    eng_map = {}
    for eng in (nc.tensor, nc.vector, nc.scalar, nc.gpsimd, nc.sync):
        eng_map[eng.engine] = eng

    f = nc.m.functions[0]
    blocks = list(f.blocks)

    def make_nop(engine_type, wait):
        eng = eng_map[engine_type]
        bi = eng.nop()
        mi = bi.ins
        mi.sync_info = bass_rust.SyncInfo(on_wait=[wait], on_update=[])
        for b in blocks:
            bl = b.instructions
            for j in range(len(bl) - 1, -1, -1):
                if bl[j] is mi:
                    del bl[j]
                    return mi
        raise RuntimeError("nop not found after emission")

    for blk in blocks:
        insts = blk.instructions
        out = []
        changed = False
        for i in insts:
            si = i.sync_info
            if si is not None and len(si.on_wait) > 1:
                waits = list(si.on_wait)
                for w in waits[:-1]:
                    out.append(make_nop(i.engine, w))
                i.sync_info = bass_rust.SyncInfo(
                    on_wait=[waits[-1]], on_update=list(si.on_update)
                )
                changed = True
            out.append(i)
        if changed:
            insts[:] = out


def _host_prep(x, qkv_w, qkv_b, proj_w, proj_b, rel_pos_x, rel_pos_y, rel_pos_z):
    """Build the shared (replicated) device arrays from the raw inputs."""
    qkv_w = np.asarray(qkv_w, np.float32)
    qkv_b = np.asarray(qkv_b, np.float32)
    proj_w = np.asarray(proj_w, np.float32)
    proj_b = np.asarray(proj_b, np.float32)
    rels = [np.asarray(r, np.float32) for r in (rel_pos_x, rel_pos_y, rel_pos_z)]

    # wqk [128, (side, pair, cc) x 128]: lhsT chunks, out cols = h0|h1
    wqk = np.zeros((128, 18, 128), np.float32)
    bqk = np.zeros((128, 6), np.float32)
    for side in range(2):
        scale = SCALE if side == 0 else 1.0
        Wm = qkv_w[:, side * C:(side + 1) * C] * scale
        bm = qkv_b[side * C:(side + 1) * C] * scale
        for p in range(NP):
            h0, h1 = 2 * p, 2 * p + 1
            blk = np.concatenate(
                [Wm[:, 64 * h0:64 * h0 + 64], Wm[:, 64 * h1:64 * h1 + 64]],
                axis=1)  # [384, 128]
            for cc in range(3):
                wqk[:, side * 9 + p * 3 + cc, :] = blk[128 * cc:128 * cc + 128, :]
            bqk[0:64, side * 3 + p] = bm[64 * h0:64 * h0 + 64]
            bqk[64:128, side * 3 + p] = bm[64 * h1:64 * h1 + 64]
    wqk = wqk.reshape(128, 18 * 128).astype(BF16)

    # v weights 65-strided with zero ones-column
    Wv3 = qkv_w[:, 2 * C:].reshape(C, NH, 64)
    wva = np.zeros((C, NH, 65), np.float32)
    wva[:, :, 0:64] = Wv3
    wv = np.ascontiguousarray(
        wva.reshape(3, 128, NH * 65).transpose(1, 0, 2).reshape(128, 3 * 390)
    ).astype(BF16)
    vb = np.zeros((NH, 65), np.float32)
    vb[:, 0:64] = qkv_b[2 * C:].reshape(NH, 64)
    vb[:, 64] = 1.0
    vb = vb.reshape(1, 390).astype(BF16)

    # proj weights pair-packed, bias broadcast
    pw = np.zeros((128, 3, C), np.float32)
    for p in range(NP):
        pw[0:64, p, :] = proj_w[64 * (2 * p):64 * (2 * p) + 64, :]
        pw[64:128, p, :] = proj_w[64 * (2 * p + 1):64 * (2 * p + 1) + 64, :]
    pw = pw.reshape(128, 3 * C).astype(BF16)
    pbb = np.broadcast_to(proj_b.reshape(1, C), (128, C)).astype(BF16)

    # rel tables, 32-wide blocks (cols 8:32 zero so the matmul fills the
    # psum rows between rel slots): rtbl8[c, (a*8+g)*32 + j] = Ra[g-j+7, c]/SCALE
    rtbl = np.zeros((64, 768), np.float32)
    for a in range(3):
        Ra = rels[a]
        for g in range(8):
            for j in range(8):
                rtbl[:, (a * 8 + g) * 32 + j] = Ra[g - j + 7, :] / SCALE
    rtbl8 = rtbl.astype(FP8)

    # static DoubleRow init tiles: stk8i has E indicators in ktile1 at
    # slots 0:8 (z-coord), 32:40 (y), 64:72 (x); everything else zero
    k_idx = np.arange(T)
    dk, hk, wk = k_idx >> 6, (k_idx >> 3) & 7, k_idx & 7
    stk8i = np.zeros((72, 2, T), np.float32)
    for cpr in range(8):
        stk8i[cpr, 1, :] = (dk == cpr)
        stk8i[32 + cpr, 1, :] = (hk == cpr)
        stk8i[64 + cpr, 1, :] = (wk == cpr)
    stk8i = np.ascontiguousarray(
        np.broadcast_to(stk8i.reshape(72, 1, 2 * T), (72, NH, 2 * T))
    ).reshape(72, NH * 2 * T).astype(FP8)
    rhq8i = np.zeros((72, NH * 2 * T), FP8)

    ones1 = np.ones((1, 128), BF16)

    return dict(
        wqk=wqk, bqk=bqk, wv=wv, vb=vb, pw=pw, pbb=pbb,
        rtbl8=rtbl8, stk8i=stk8i, rhq8i=rhq8i, ones1=ones1,
    )


LAST_EXEC_NS = None


def kernel(**inputs) -> np.ndarray:
    global LAST_EXEC_NS
    from concourse.bass_utils import run_bass_kernel_spmd

    if "nc" not in _CACHE:
        _CACHE["nc"] = _build_nc()
    nc = _CACHE["nc"]

    x = np.asarray(inputs["x"], np.float32)
    shared = _host_prep(
        x, inputs["qkv_w"], inputs["qkv_b"], inputs["proj_w"], inputs["proj_b"],
        inputs["rel_pos_x"], inputs["rel_pos_y"], inputs["rel_pos_z"],
    )

    # window gather indices within a [2048, C] shard (4 windows x 512 tokens)
    t = np.arange(T)
    z, yy, xx = t >> 6, (t >> 3) & 7, t & 7
    rows_w = np.stack([
        256 * z + 16 * (8 * (w >> 1) + yy) + (8 * (w & 1) + xx) for w in range(4)
    ])  # [4, 512]

    in_maps = []
    for i in range(NCORES):
        b, half = i // 2, i % 2
        m = dict(shared)
        xs = x[b, half * 2048:(half + 1) * 2048, :]          # [2048, C]
        xw = xs[rows_w, :]                                    # [4, 512, C]
        xt4 = xw.transpose(0, 2, 1).reshape(4, 3, 128, T)
        m["xt_sh"] = np.ascontiguousarray(
            xt4.transpose(0, 2, 1, 3)
        ).astype(BF16)                                        # [4, 128, 3, 512]
        in_maps.append(m)

    trace = bool(os.environ.get("KERNEL_TRACE"))
    res = run_bass_kernel_spmd(
        nc, in_maps, core_ids=list(range(NCORES)), trace=trace,
    )
    LAST_EXEC_NS = res.exec_time_ns

    out = np.empty((B, N, C), np.float32)
    for i in range(NCORES):
        b, half = i // 2, i % 2
        ow = res.results[i]["out_w"]                          # [4, 512, C]
        sh = np.empty((2048, C), np.float32)
        sh[rows_w.reshape(-1), :] = ow.reshape(4 * T, C)
        out[b, half * 2048:(half + 1) * 2048, :] = sh
    return out.reshape(B, N, C)


# revision 33
# speedup vs baseline: 1.2959x; 1.0028x over previous
"""Trainium2 Bass kernel for windowed 3D attention with decomposed rel-pos bias.

Problem: B=4, N=4096 (16^3), C=384, window 8^3=512 tokens, 6 heads x 64 dim.
Sharding: 8 cores, data-parallel over 32 windows (4 per core). Core i takes
batch b=i//2, z-half h=i%2 -> a contiguous [2048, 384] slice of x holding 4
windows.

Device-side per window (v2, fp8-DoubleRow S^T):
  - q/k projections 2-head-packed: one [128,512] PSUM tile per head-pair per
    contraction chunk (out rows 0:64 = even head, 64:128 = odd head).
  - S^T contraction operands stored fp8e4m3 in DoubleRow layout [64,2,512]:
    ktile0 = 64 channel rows, ktile1 rows 0:24 = rel-pos terms (q side) /
    one-hot E indicators (k side, static), rows 24:64 zero. One DoubleRow
    matmul per (head, k-chunk) computes S^T at 0.5 cycles/col.
  - exp on ACT (f32 PSUM -> bf16 SBUF), attn@V in bf16 with a ones column
    producing the softmax denominator row.
  - normalization pair-packed: recip denom rows, rank-1 broadcast matmuls,
    one tensor-tensor multiply per pair.
  - output projection contracts a full head-pair (128 rows) per matmul;
    v/proj biases folded into PSUM-evac tensor-tensor adds against
    host-broadcast bias tiles (no bias matmuls).
"""

import os
import numpy as np
import ml_dtypes

BF16 = np.float16            # host array dtype for mybir float16 params
FP8 = ml_dtypes.float8_e4m3fn

B, N, C = 4, 4096, 384
WS, NH, HD = 8, 6, 64
T = WS * WS * WS  # 512
SCALE = HD ** -0.5
NCORES = 8
NP = NH // 2  # head pairs

_CACHE = {}


def _build_nc():
    import concourse.bass as bass
    import concourse.tile as tile
    import concourse.mybir as mybir
    from contextlib import ExitStack

    f32 = mybir.dt.float32
    bf16 = mybir.dt.float16
    fp8 = mybir.dt.float8e4
    Exp = mybir.ActivationFunctionType.Exp
    Copy = mybir.ActivationFunctionType.Copy
    Ident = mybir.ActivationFunctionType.Identity
    add = mybir.AluOpType.add
    mult = mybir.AluOpType.mult
    DR = mybir.MatmulPerfMode.DoubleRow

    nc = bass.Bass("TRN2")

    x_d = nc.declare_dram_parameter("xt_sh", [4, 128, 3, T], bf16, isOutput=False)
    wqk_d = nc.declare_dram_parameter("wqk", [128, 18 * 128], bf16, isOutput=False)
    bqk_d = nc.declare_dram_parameter("bqk", [128, 6], f32, isOutput=False)
    wv_d = nc.declare_dram_parameter("wv", [128, 3 * 390], bf16, isOutput=False)
    vb_d = nc.declare_dram_parameter("vb", [1, 390], bf16, isOutput=False)
    pw_d = nc.declare_dram_parameter("pw", [128, 3 * 384], bf16, isOutput=False)
    pbb_d = nc.declare_dram_parameter("pbb", [128, 384], bf16, isOutput=False)
    rtbl_d = nc.declare_dram_parameter("rtbl8", [64, 768], fp8, isOutput=False)
    stki_d = nc.declare_dram_parameter("stk8i", [72, NH * 2 * T], fp8, isOutput=False)
    rhqi_d = nc.declare_dram_parameter("rhq8i", [72, NH * 2 * T], fp8, isOutput=False)
    ones_d = nc.declare_dram_parameter("ones1", [1, 128], bf16, isOutput=False)
    out_d = nc.declare_dram_parameter("out_w", [4, T, C], f32, isOutput=True)

    with tile.TileContext(nc) as tc, ExitStack() as ctx:
        const = ctx.enter_context(tc.tile_pool(name="const", bufs=1))
        xt_p = ctx.enter_context(tc.tile_pool(name="xt", bufs=3))
        vaug_p = ctx.enter_context(tc.tile_pool(name="vaug", bufs=3))
        pp_p = ctx.enter_context(tc.tile_pool(name="pp", bufs=6))
        pf_p = ctx.enter_context(tc.tile_pool(name="pf", bufs=9))
        rd_p = ctx.enter_context(tc.tile_pool(name="rd", bufs=6))
        ost_p = ctx.enter_context(tc.tile_pool(name="ost", bufs=6))
        qkp = ctx.enter_context(tc.tile_pool(name="qkp", bufs=2, space="PSUM"))
        stp = ctx.enter_context(tc.tile_pool(name="stp", bufs=2, space="PSUM"))
        mip = ctx.enter_context(tc.tile_pool(name="mip", bufs=2, space="PSUM"))

        # --- first window's x plus constants, ordered by first use; the big
        # weight / init tensors load on the ACT queue in parallel with SP ---
        xt0 = xt_p.tile([128, 3, T], bf16)
        nc.sync.dma_start(out=xt0[:, :, :], in_=x_d[0, :, :, :])
        wv_sb = const.tile([128, 3 * 390], bf16)
        nc.sync.dma_start(out=wv_sb, in_=wv_d[:, :])
        vb_sb = const.tile([1, 390], bf16)
        nc.sync.dma_start(out=vb_sb, in_=vb_d[:, :])
        bqk_sb = const.tile([128, 6], f32)
        nc.sync.dma_start(out=bqk_sb, in_=bqk_d[:, :])
        rtbl_sb = const.tile([64, 768], fp8)
        nc.sync.dma_start(out=rtbl_sb, in_=rtbl_d[:, :])
        ones_sb = const.tile([1, 128], bf16)
        nc.sync.dma_start(out=ones_sb, in_=ones_d[:, :])

        wqk_sb = const.tile([128, 18 * 128], bf16)
        nc.scalar.dma_start(out=wqk_sb, in_=wqk_d[:, :])
        pw_sb = const.tile([128, 3 * 384], bf16)
        nc.scalar.dma_start(out=pw_sb, in_=pw_d[:, :])
        pbb_sb = const.tile([128, 384], bf16)
        nc.scalar.dma_start(out=pbb_sb, in_=pbb_d[:, :])

        # persistent DoubleRow operand tiles [72, 2, 512] fp8 per head:
        # ktile0 rows 0:64 = channels (rows 64:72 zero-pad), ktile1 = rel/E at
        # slots 0:8 / 32:40 / 64:72 with zeros between (legal PSUM out bases)
        stk8bs, rhq8bs = [], []
        for bi in range(2):
            sb = const.tile([72, NH, 2, T], fp8, name=f"stk8b{bi}", tag=f"stk8b{bi}")
            nc.sync.dma_start(
                out=sb.rearrange("p h t x -> p (h t x)"), in_=stki_d[:, :])
            rb = const.tile([72, NH, 2, T], fp8, name=f"rhq8b{bi}", tag=f"rhq8b{bi}")
            nc.sync.dma_start(
                out=rb.rearrange("p h t x -> p (h t x)"), in_=rhqi_d[:, :])
            stk8bs.append(sb)
            rhq8bs.append(rb)

        def wq_ap(side, pair, cc):
            i = (side * 9 + pair * 3 + cc) * 128
            return wqk_sb[:, i:i + 128]

        for w in range(4):
            stk8b = stk8bs[w % 2]
            rhq8b = rhq8bs[w % 2]
            if w == 0:
                xt = xt0
            else:
                xt = xt_p.tile([128, 3, T], bf16)
                nc.sync.dma_start(out=xt[:, :, :], in_=x_d[w, :, :, :])

            # --- v in token layout, 65-strided, bias+ones via Pool TT-add ---
            va = vaug_p.tile([128, 4, 390], bf16)
            for ct in range(4):
                vp = mip.tile([128, 512], f32, tag="mi", name="vp")
                for cc in range(3):
                    nc.tensor.matmul(
                        vp[:, 0:390],
                        lhsT=xt[:, cc, 128 * ct:128 * ct + 128],
                        rhs=wv_sb[:, 390 * cc:390 * cc + 390],
                        start=(cc == 0), stop=False,
                    )
                nc.tensor.matmul(
                    vp[:, 0:390], lhsT=ones_sb[0:1, :], rhs=vb_sb[0:1, :],
                    start=False, stop=True,
                )
                nc.scalar.activation(
                    out=va[:, ct, :], in_=vp[:, 0:390], func=Copy)

            # --- q/k projections, 2-head-packed; fp8 DoubleRow operands ---
            for p in range(NP):
                h0, h1 = 2 * p, 2 * p + 1
                qp = qkp.tile([128, 512], f32, tag="qk", name="qp")
                for cc in range(3):
                    nc.tensor.matmul(
                        qp[:, :], lhsT=wq_ap(0, p, cc), rhs=xt[:, cc, :],
                        start=(cc == 0), stop=(cc == 2),
                    )
                nc.vector.tensor_scalar(
                    out=rhq8b[0:64, h0, 0, :], in0=qp[0:64, :],
                    scalar1=bqk_sb[0:64, p:p + 1], scalar2=None, op0=add,
                )
                nc.vector.tensor_scalar(
                    out=rhq8b[0:64, h1, 0, :], in0=qp[64:128, :],
                    scalar1=bqk_sb[64:128, p:p + 1], scalar2=None, op0=add,
                )
                # rel-pos terms for each head of the pair; each (a, g) matmul
                # writes a 32-row block (8 rel rows + 24 table-zero rows) so
                # psum rows 0:96 are all written and one evac covers 0:72
                for h in (h0, h1):
                    rp = qkp.tile([128, 512], f32, tag="qk", name="rp")
                    rq3 = rhq8b.rearrange(
                        "p h t (z y x) -> p h t z y x", z=8, y=8, x=8)
                    rp3 = rp.rearrange("p (z y x) -> p z y x", z=8, y=8, x=8)
                    for a in range(3):
                        for g in range(8):
                            lhsT = rtbl_sb[:, (a * 8 + g) * 32:(a * 8 + g + 1) * 32]
                            if a == 0:
                                rhs_ap = rq3[0:64, h, 0, g, :, :]
                                out_ap = rp[0:32, 64 * g:64 * g + 64]
                            elif a == 1:
                                rhs_ap = rq3[0:64, h, 0, :, g, :]
                                out_ap = rp3[32:64, :, g, :]
                            else:
                                rhs_ap = rq3[0:64, h, 0, :, :, g]
                                out_ap = rp3[64:96, :, :, g]
                            nc.tensor.matmul(
                                out_ap, lhsT=lhsT, rhs=rhs_ap,
                                start=(g == 0), stop=(g == 7),
                            )
                    nc.vector.tensor_copy(rhq8b[0:72, h, 1, :], rp[0:72, :])

                kp = qkp.tile([128, 512], f32, tag="qk", name="kp")
                for cc in range(3):
                    nc.tensor.matmul(
                        kp[:, :], lhsT=wq_ap(1, p, cc), rhs=xt[:, cc, :],
                        start=(cc == 0), stop=(cc == 2),
                    )
                nc.scalar.activation(
                    out=stk8b[0:64, h0, 0, :], in_=kp[0:64, :],
                    func=Ident, bias=bqk_sb[0:64, 3 + p:4 + p],
                )
                nc.scalar.activation(
                    out=stk8b[0:64, h1, 0, :], in_=kp[64:128, :],
                    func=Ident, bias=bqk_sb[64:128, 3 + p:4 + p],
                )

            # --- attention per pair ---
            pf_list = []
            for p in range(NP):
                rdr = [rd_p.tile([1, T], bf16, tag="rdr", name="rdr")
                       for _ in range(2)]
                ots = []
                for j, h in enumerate((2 * p, 2 * p + 1)):
                    ot = mip.tile([128, 512], f32, tag="mi", name="ot")
                    for half in range(2):
                        stt = stp.tile([128, 1024], f32, tag="stt", name="stt")
                        for jj in range(2):
                            kc = 2 * half + jj
                            nc.tensor.matmul(
                                stt[:, 512 * jj:512 * jj + 512],
                                lhsT=stk8b[:, h, :, 128 * kc:128 * kc + 128],
                                rhs=rhq8b[:, h, :, :],
                                start=True, stop=True, perf_mode=DR,
                            )
                        pp = pp_p.tile([128, 1024], bf16)
                        nc.scalar.activation(out=pp[:, :], in_=stt[:, :], func=Exp)
                        for jj in range(2):
                            kc = 2 * half + jj
                            nc.tensor.matmul(
                                ot[0:65, :],
                                lhsT=va[:, kc, 65 * h:65 * h + 65],
                                rhs=pp[:, 512 * jj:512 * jj + 512],
                                start=(kc == 0), stop=(kc == 3),
                            )
                    with nc.allow_low_precision(reason="softmax denom recip"):
                        nc.vector.reciprocal(rdr[j][0:1, :], ot[64:65, :])
                    ots.append(ot)
                # pair-packed normalization
                bc = stp.tile([128, 512], f32, tag="stt", name="bc")
                nc.tensor.matmul(
                    bc[0:64, :], lhsT=ones_sb[0:1, 0:64], rhs=rdr[0][0:1, :],
                    start=True, stop=True,
                )
                nc.tensor.matmul(
                    bc[64:128, :], lhsT=ones_sb[0:1, 0:64], rhs=rdr[1][0:1, :],
                    start=True, stop=True,
                )
                rdrb = pf_p.tile([128, T], bf16, tag="bcs", name="rdrb")
                nc.vector.tensor_copy(rdrb[:, :], bc[:, :])
                pf = pf_p.tile([128, T], bf16, tag="pf", name="pf")
                nc.vector.tensor_tensor(
                    out=pf[0:64, :], in0=ots[0][0:64, :], in1=rdrb[0:64, :],
                    op=mult)
                nc.vector.tensor_tensor(
                    out=pf[64:128, :], in0=ots[1][0:64, :], in1=rdrb[64:128, :],
                    op=mult)
                pf_list.append(pf)

            # --- output projection: full head-pair contract per matmul ---
            for qc in range(4):
                prj = mip.tile([128, 512], f32, tag="mi", name="prj")
                for p in range(NP):
                    nc.tensor.matmul(
                        prj[:, 0:C],
                        lhsT=pf_list[p][:, 128 * qc:128 * qc + 128],
                        rhs=pw_sb[:, C * p:C * p + C],
                        start=(p == 0), stop=(p == 2),
                    )
                ost = ost_p.tile([128, C], f32)
                nc.vector.tensor_tensor(
                    out=ost[:, :], in0=prj[:, 0:C], in1=pbb_sb[:, :], op=add)
                nc.sync.dma_start(
                    out=out_d[w, 128 * qc:128 * qc + 128, :], in_=ost[:, :])

    _fix_multiwait(nc)
    return nc


def _fix_multiwait(nc):
    """Walrus in this container rejects instructions with >1 sync wait.
    Move extra waits onto same-engine NOPs inserted just before."""
    import bass_rust

    eng_map = {}
    for eng in (nc.tensor, nc.vector, nc.scalar, nc.gpsimd, nc.sync):
        eng_map[eng.engine] = eng

    f = nc.m.functions[0]
    blocks = list(f.blocks)

    def make_nop(engine_type, wait):
        eng = eng_map[engine_type]
        bi = eng.nop()
        mi = bi.ins
        mi.sync_info = bass_rust.SyncInfo(on_wait=[wait], on_update=[])
        for b in blocks:
            bl = b.instructions
            for j in range(len(bl) - 1, -1, -1):
                if bl[j] is mi:
                    del bl[j]
                    return mi
        raise RuntimeError("nop not found after emission")

    for blk in blocks:
        insts = blk.instructions
        out = []
        changed = False
        for i in insts:
            si = i.sync_info
            if si is not None and len(si.on_wait) > 1:
                waits = list(si.on_wait)
                for w in waits[:-1]:
                    out.append(make_nop(i.engine, w))
                i.sync_info = bass_rust.SyncInfo(
                    on_wait=[waits[-1]], on_update=list(si.on_update)
                )
                changed = True
            out.append(i)
        if changed:
            insts[:] = out


def _host_prep(x, qkv_w, qkv_b, proj_w, proj_b, rel_pos_x, rel_pos_y, rel_pos_z):
    """Build the shared (replicated) device arrays from the raw inputs."""
    qkv_w = np.asarray(qkv_w, np.float32)
    qkv_b = np.asarray(qkv_b, np.float32)
    proj_w = np.asarray(proj_w, np.float32)
    proj_b = np.asarray(proj_b, np.float32)
    rels = [np.asarray(r, np.float32) for r in (rel_pos_x, rel_pos_y, rel_pos_z)]

    # wqk [128, (side, pair, cc) x 128]: lhsT chunks, out cols = h0|h1
    wqk = np.zeros((128, 18, 128), np.float32)
    bqk = np.zeros((128, 6), np.float32)
    for side in range(2):
        scale = SCALE if side == 0 else 1.0
        Wm = qkv_w[:, side * C:(side + 1) * C] * scale
        bm = qkv_b[side * C:(side + 1) * C] * scale
        for p in range(NP):
            h0, h1 = 2 * p, 2 * p + 1
            blk = np.concatenate(
                [Wm[:, 64 * h0:64 * h0 + 64], Wm[:, 64 * h1:64 * h1 + 64]],
                axis=1)  # [384, 128]
            for cc in range(3):
                wqk[:, side * 9 + p * 3 + cc, :] = blk[128 * cc:128 * cc + 128, :]
            bqk[0:64, side * 3 + p] = bm[64 * h0:64 * h0 + 64]
            bqk[64:128, side * 3 + p] = bm[64 * h1:64 * h1 + 64]
    wqk = wqk.reshape(128, 18 * 128).astype(BF16)

    # v weights 65-strided with zero ones-column
    Wv3 = qkv_w[:, 2 * C:].reshape(C, NH, 64)
    wva = np.zeros((C, NH, 65), np.float32)
    wva[:, :, 0:64] = Wv3
    wv = np.ascontiguousarray(
        wva.reshape(3, 128, NH * 65).transpose(1, 0, 2).reshape(128, 3 * 390)
    ).astype(BF16)
    vb = np.zeros((NH, 65), np.float32)
    vb[:, 0:64] = qkv_b[2 * C:].reshape(NH, 64)
    vb[:, 64] = 1.0
    vb = vb.reshape(1, 390).astype(BF16)

    # proj weights pair-packed, bias broadcast
    pw = np.zeros((128, 3, C), np.float32)
    for p in range(NP):
        pw[0:64, p, :] = proj_w[64 * (2 * p):64 * (2 * p) + 64, :]
        pw[64:128, p, :] = proj_w[64 * (2 * p + 1):64 * (2 * p + 1) + 64, :]
    pw = pw.reshape(128, 3 * C).astype(BF16)
    pbb = np.broadcast_to(proj_b.reshape(1, C), (128, C)).astype(BF16)

    # rel tables, 32-wide blocks (cols 8:32 zero so the matmul fills the
    # psum rows between rel slots): rtbl8[c, (a*8+g)*32 + j] = Ra[g-j+7, c]/SCALE
    rtbl = np.zeros((64, 768), np.float32)
    for a in range(3):
        Ra = rels[a]
        for g in range(8):
            for j in range(8):
                rtbl[:, (a * 8 + g) * 32 + j] = Ra[g - j + 7, :] / SCALE
    rtbl8 = rtbl.astype(FP8)

    # static DoubleRow init tiles: stk8i has E indicators in ktile1 at
    # slots 0:8 (z-coord), 32:40 (y), 64:72 (x); everything else zero
    k_idx = np.arange(T)
    dk, hk, wk = k_idx >> 6, (k_idx >> 3) & 7, k_idx & 7
    stk8i = np.zeros((72, 2, T), np.float32)
    for cpr in range(8):
        stk8i[cpr, 1, :] = (dk == cpr)
        stk8i[32 + cpr, 1, :] = (hk == cpr)
        stk8i[64 + cpr, 1, :] = (wk == cpr)
    stk8i = np.ascontiguousarray(
        np.broadcast_to(stk8i.reshape(72, 1, 2 * T), (72, NH, 2 * T))
    ).reshape(72, NH * 2 * T).astype(FP8)
    rhq8i = np.zeros((72, NH * 2 * T), FP8)

    ones1 = np.ones((1, 128), BF16)

    return dict(
        wqk=wqk, bqk=bqk, wv=wv, vb=vb, pw=pw, pbb=pbb,
        rtbl8=rtbl8, stk8i=stk8i, rhq8i=rhq8i, ones1=ones1,
    )


LAST_EXEC_NS = None


def kernel(**inputs) -> np.ndarray:
    global LAST_EXEC_NS
    from concourse.bass_utils import run_bass_kernel_spmd

    if "nc" not in _CACHE:
        _CACHE["nc"] = _build_nc()
    nc = _CACHE["nc"]

    x = np.asarray(inputs["x"], np.float32)
    shared = _host_prep(
        x, inputs["qkv_w"], inputs["qkv_b"], inputs["proj_w"], inputs["proj_b"],
        inputs["rel_pos_x"], inputs["rel_pos_y"], inputs["rel_pos_z"],
    )

    # window gather indices within a [2048, C] shard (4 windows x 512 tokens)
    t = np.arange(T)
    z, yy, xx = t >> 6, (t >> 3) & 7, t & 7
    rows_w = np.stack([
        256 * z + 16 * (8 * (w >> 1) + yy) + (8 * (w & 1) + xx) for w in range(4)
    ])  # [4, 512]

    in_maps = []
    for i in range(NCORES):
        b, half = i // 2, i % 2
        m = dict(shared)
        xs = x[b, half * 2048:(half + 1) * 2048, :]          # [2048, C]
        xw = xs[rows_w, :]                                    # [4, 512, C]
        xt4 = xw.transpose(0, 2, 1).reshape(4, 3, 128, T)
        m["xt_sh"] = np.ascontiguousarray(
            xt4.transpose(0, 2, 1, 3)
        ).astype(BF16)                                        # [4, 128, 3, 512]
        in_maps.append(m)

    trace = bool(os.environ.get("KERNEL_TRACE"))
    res = run_bass_kernel_spmd(
        nc, in_maps, core_ids=list(range(NCORES)), trace=trace,
    )
    LAST_EXEC_NS = res.exec_time_ns

    out = np.empty((B, N, C), np.float32)
    for i in range(NCORES):
        b, half = i // 2, i % 2
        ow = res.results[i]["out_w"]                          # [4, 512, C]
        sh = np.empty((2048, C), np.float32)
        sh[rows_w.reshape(-1), :] = ow.reshape(4 * T, C)
        out[b, half * 2048:(half + 1) * 2048, :] = sh
    return out.reshape(B, N, C)
